# revision 4
# baseline (speedup 1.0000x reference)
"""AdaptiveSparseMoE (B=4,S=2048,D=1024,E=4,K=2,F=4096) on 8 TRN2 NeuronCores.

Strategy: pure data parallelism. B*S = 8192 tokens are split into 8 shards of
1024 tokens; every core holds all 4 experts' weights (bf16) and computes
  gate = x @ Wg + bg            (fp32 on TensorE -- top-2 selection must
                                 match the reference bit-for-bit on near-ties)
  mask = one-hot(top2(gate))    (Max8 + match_replace on VectorE: exact
                                 jax.lax.top_k tie semantics)
  h_e  = silu(x @ W1[e] + b1[e])   (bf16 matmul, Silu on ScalarE)
  y    = sum_e mask[:,e] * (h_e @ W2[e]) + mask @ b2
with the per-expert mask applied during PSUM eviction via a fused
(psum * mask) + acc scalar_tensor_tensor on VectorE. No collectives.

Host-side work is layout only: token-shard transpose of x (the TensorE wants
the contraction dim on partitions), bf16 cast of the expert weights, and a
[128,32] re-tile of b1 so its per-partition bias DMA is contiguous.
"""

import numpy as np

B, S, D, E, F = 4, 2048, 1024, 4, 4096
NCORES = 8
TOK = (B * S) // NCORES          # 1024 tokens per core
KT = D // 128                    # 8 contraction tiles for D
FT = F // 128                    # 32 f-tiles
FO = 16                          # W1 streamed in FO chunks of F
FCH = F // FO                    # 512 F columns per chunk
TT = TOK // 128                  # 8 token tiles
NEG = -1.0e30

_cache: dict = {}


def _build():
    import concourse.bass as bass
    import concourse.mybir as mybir
    import concourse.tile as tile
    from concourse import bacc
    from concourse.masks import make_identity

    fp32 = mybir.dt.float32
    bf16 = mybir.dt.bfloat16
    AF = mybir.ActivationFunctionType
    ALU = mybir.AluOpType

    nc = bacc.Bacc()
    xTf = nc.declare_dram_parameter("xTf", [D, TOK], fp32, isOutput=False)
    xTb = nc.declare_dram_parameter("xTb", [D, TOK], bf16, isOutput=False)
    wg = nc.declare_dram_parameter("wg", [D, E], fp32, isOutput=False)
    bg = nc.declare_dram_parameter("bg", [E, 1], fp32, isOutput=False)
    w1 = nc.declare_dram_parameter("w1", [E, D, F], bf16, isOutput=False)
    b1t = nc.declare_dram_parameter("b1t", [E, 128, FT], fp32, isOutput=False)
    w2 = nc.declare_dram_parameter("w2", [E, F, D], bf16, isOutput=False)
    b2 = nc.declare_dram_parameter("b2", [E, D], fp32, isOutput=False)
    out = nc.declare_dram_parameter("out", [TOK, D], fp32, isOutput=True)

    with tile.TileContext(nc) as tc:
        with (
            tc.tile_pool(name="const", bufs=1) as cpool,
            tc.tile_pool(name="big", bufs=1) as big,
            tc.tile_pool(name="w1p", bufs=2) as w1p,
            tc.tile_pool(name="xs", bufs=1) as xsp,
            tc.tile_pool(name="small", bufs=2) as sp,
        ):
            # ---- constants ----
            id4 = cpool.tile([4, 4], fp32)
            make_identity(nc, id4)
            id128 = cpool.tile([128, 128], fp32)
            make_identity(nc, id128)
            wg_sb = cpool.tile([128, KT, E], fp32)
            for k in range(KT):
                nc.sync.dma_start(wg_sb[:, k, :], wg[128 * k:128 * (k + 1), :])
            bg_sb = cpool.tile([E, 1], fp32)
            nc.sync.dma_start(bg_sb[:], bg[:, :])
            b2_sb = cpool.tile([E, D], fp32)
            nc.sync.dma_start(b2_sb[:], b2[:, :])
            b1_sb = cpool.tile([128, E, FT], fp32)
            for e in range(E):
                nc.sync.dma_start(b1_sb[:, e, :], b1t[e, :, :])

            # ---- resident activations ----
            xb = big.tile([128, KT, TOK], bf16)          # x^T in bf16, 16KB/part
            for k in range(KT):
                nc.sync.dma_start(xb[:, k, :], xTb[128 * k:128 * (k + 1), :])
            h_sb = big.tile([128, FT, TOK], bf16)        # full silu(x@W1[e]), 64KB
            y_acc = big.tile([128, TT, D], fp32)         # output accumulator, 32KB
            gate8 = big.tile([128, TT, 8], fp32)
            mask = big.tile([128, TT, E], fp32)
            maskT = big.tile([E, TOK], fp32)

            # ---- phase 0: gating (fp32) ----
            with tc.tile_pool(name="gps", bufs=1, space="PSUM") as gps, \
                 tc.tile_pool(name="trp", bufs=2, space="PSUM") as trp:
                pg = gps.tile([E, TOK], fp32)            # gate^T, 2 banks
                for k in range(KT):
                    xf = xsp.tile([128, TOK], fp32)
                    nc.sync.dma_start(xf[:], xTf[128 * k:128 * (k + 1), :])
                    for h in range(2):
                        nc.tensor.matmul(
                            pg[:, 512 * h:512 * (h + 1)],
                            wg_sb[:, k, :],
                            xf[:, 512 * h:512 * (h + 1)],
                            start=(k == 0), stop=(k == KT - 1),
                        )
                gT = big.tile([E, TOK], fp32, tag="gT")
                nc.scalar.activation(gT[:], pg[:], AF.Identity, bias=bg_sb[:])

                nc.vector.memset(gate8[:], NEG)
                for t in range(TT):
                    ptr = trp.tile([128, E], fp32, tag="ptr")
                    nc.tensor.transpose(ptr[:], gT[:, 128 * t:128 * (t + 1)], id4[:])
                    nc.vector.tensor_copy(gate8[:, t, 0:E], ptr[:])
                for t in range(TT):
                    m8 = sp.tile([128, 8], fp32, tag="m8")
                    nc.vector.max(m8[:], gate8[:, t, :])
                    nc.vector.memset(m8[:, 2:8], NEG)
                    rep = sp.tile([128, 8], fp32, tag="rep")
                    nc.vector.match_replace(rep[:], m8[:], gate8[:, t, :], NEG)
                    nc.vector.tensor_tensor(
                        mask[:, t, :], gate8[:, t, 0:E], rep[:, 0:E], ALU.is_gt)
                    pmt = trp.tile([E, 128], fp32, tag="pmt")
                    nc.tensor.transpose(pmt[:], mask[:, t, :], id128[:])
                    nc.vector.tensor_copy(maskT[:, 128 * t:128 * (t + 1)], pmt[:])

            # ---- expert phase ----
            with tc.tile_pool(name="hps", bufs=2, space="PSUM") as hps, \
                 tc.tile_pool(name="yps", bufs=2, space="PSUM") as yps:
                # init y_acc with the (mask @ b2) term
                for t in range(TT):
                    pb = yps.tile([128, D], fp32, tag="y")
                    for h in range(2):
                        nc.tensor.matmul(
                            pb[:, 512 * h:512 * (h + 1)],
                            maskT[:, 128 * t:128 * (t + 1)],
                            b2_sb[:, 512 * h:512 * (h + 1)],
                            start=True, stop=True)
                    nc.scalar.activation(y_acc[:, t, :], pb[:], AF.Identity)

                for e in range(E):
                    w2t = big.tile([128, FT, D], bf16, tag="w2t")  # 64KB/part
                    for f in range(FT):
                        nc.sync.dma_start(
                            w2t[:, f, :], w2[e, 128 * f:128 * (f + 1), :])
                    # mm1 + silu: h^T[f,:] for all F
                    for fo in range(FO):
                        w1t = w1p.tile([128, KT, FCH], bf16, tag="w1t")
                        for k in range(KT):
                            nc.sync.dma_start(
                                w1t[:, k, :],
                                w1[e, 128 * k:128 * (k + 1),
                                   FCH * fo:FCH * (fo + 1)])
                        for fi in range(FCH // 128):
                            ft = fo * (FCH // 128) + fi
                            ph = hps.tile([128, TOK], fp32, tag="h")
                            for k in range(KT):
                                lw = w1t[:, k, 128 * fi:128 * (fi + 1)]
                                for h in range(2):
                                    nc.tensor.matmul(
                                        ph[:, 512 * h:512 * (h + 1)],
                                        lw,
                                        xb[:, k, 512 * h:512 * (h + 1)],
                                        start=(k == 0), stop=(k == KT - 1))
                            nc.scalar.activation(
                                h_sb[:, ft, :], ph[:], AF.Silu,
                                bias=b1_sb[:, e, ft:ft + 1])
                    # mm2 + masked accumulate
                    for t in range(TT):
                        py = yps.tile([128, D], fp32, tag="y")
                        for f in range(FT):
                            lh = h_sb[:, f, 128 * t:128 * (t + 1)]
                            for h in range(2):
                                nc.tensor.matmul(
                                    py[:, 512 * h:512 * (h + 1)],
                                    lh,
                                    w2t[:, f, 512 * h:512 * (h + 1)],
                                    start=(f == 0), stop=(f == FT - 1))
                        nc.vector.scalar_tensor_tensor(
                            y_acc[:, t, :], py[:], mask[:, t, e:e + 1],
                            y_acc[:, t, :], ALU.mult, ALU.add)

            for t in range(TT):
                nc.sync.dma_start(out[128 * t:128 * (t + 1), :], y_acc[:, t, :])

    nc.finalize()
    return nc


def _get_nc():
    if "nc" not in _cache:
        _cache["nc"] = _build()
    return _cache["nc"]


def kernel(x, Wg, bg, W1, b1, W2, b2):
    import ml_dtypes
    from concourse.bass_utils import run_bass_kernel_spmd

    nc = _get_nc()
    bf = ml_dtypes.bfloat16

    x = np.asarray(x, dtype=np.float32).reshape(B * S, D)
    Wg = np.asarray(Wg, dtype=np.float32)
    bg_c = np.ascontiguousarray(np.asarray(bg, np.float32).reshape(E, 1))
    W1b = np.ascontiguousarray(np.asarray(W1, np.float32)).astype(bf)
    W2b = np.ascontiguousarray(np.asarray(W2, np.float32)).astype(bf)
    # b1 [E,F] -> [E,128,FT] so each expert's per-partition bias column loads
    # with one contiguous DMA: b1t[e, p, j] = b1[e, j*128 + p]
    b1t = np.ascontiguousarray(
        np.asarray(b1, np.float32).reshape(E, FT, 128).transpose(0, 2, 1))
    b2_c = np.ascontiguousarray(np.asarray(b2, np.float32))

    in_maps = []
    for c in range(NCORES):
        xs = x[c * TOK:(c + 1) * TOK, :]          # [TOK, D]
        xT = np.ascontiguousarray(xs.T)           # [D, TOK] fp32
        in_maps.append({
            "xTf": xT,
            "xTb": xT.astype(bf),
            "wg": Wg,
            "bg": bg_c,
            "w1": W1b,
            "b1t": b1t,
            "w2": W2b,
            "b2": b2_c,
        })

    res = run_bass_kernel_spmd(nc, in_maps, core_ids=list(range(NCORES)),
                               **_cache.get("run_kwargs", {}))
    _cache["last_result"] = res
    out = np.concatenate([np.asarray(res.results[c]["out"])
                          for c in range(NCORES)], axis=0)
    return out.reshape(B, S, D).astype(np.float32)


# revision 9
# speedup vs baseline: 1.1690x; 1.1690x over previous
"""AdaptiveSparseMoE (B=4,S=2048,D=1024,E=4,K=2,F=4096) on 8 TRN2 NeuronCores.

Strategy: pure data parallelism. B*S = 8192 tokens are split into 8 shards of
1024 tokens; every core holds all 4 experts' weights (bf16) and computes
  gate = x @ Wg + bg            (fp32 on TensorE -- top-2 selection must
                                 match the reference bit-for-bit on near-ties)
  mask = one-hot(top2(gate))    (Max8 + match_replace on VectorE: exact
                                 jax.lax.top_k tie semantics)
  h_e  = silu(x @ W1[e] + b1[e])   (bf16 matmul, Silu on ScalarE)
  y    = sum_e mask[:,e] * (h_e @ W2[e]) + mask @ b2
with the per-expert mask applied during PSUM eviction via a fused
(psum * mask) + acc scalar_tensor_tensor on VectorE. No collectives.

Host-side work is layout only: token-shard transpose of x (the TensorE wants
the contraction dim on partitions), bf16 cast of the expert weights, and a
[128,32] re-tile of b1 so its per-partition bias DMA is contiguous.
"""

import numpy as np

B, S, D, E, F = 4, 2048, 1024, 4, 4096
NCORES = 8
TOK = (B * S) // NCORES          # 1024 tokens per core
KT = D // 128                    # 8 contraction tiles for D
FT = F // 128                    # 32 f-tiles
FO = 16                          # W1 streamed in FO chunks of F
FCH = F // FO                    # 512 F columns per chunk
TT = TOK // 128                  # 8 token tiles
NEG = -1.0e30

_cache: dict = {}


def _build():
    import concourse.bass as bass
    import concourse.mybir as mybir
    import concourse.tile as tile
    from concourse import bacc
    from concourse.masks import make_identity

    fp32 = mybir.dt.float32
    bf16 = mybir.dt.bfloat16
    AF = mybir.ActivationFunctionType
    ALU = mybir.AluOpType

    nc = bacc.Bacc()
    xTf = nc.declare_dram_parameter("xTf", [D, TOK], fp32, isOutput=False)
    xTb = nc.declare_dram_parameter("xTb", [D, TOK], bf16, isOutput=False)
    wg = nc.declare_dram_parameter("wg", [D, E], fp32, isOutput=False)
    bg = nc.declare_dram_parameter("bg", [E, 1], fp32, isOutput=False)
    w1 = nc.declare_dram_parameter("w1", [E, D, F], bf16, isOutput=False)
    b1t = nc.declare_dram_parameter("b1t", [E, 128, FT], fp32, isOutput=False)
    w2 = nc.declare_dram_parameter("w2", [E, F, D], bf16, isOutput=False)
    b2 = nc.declare_dram_parameter("b2", [E, D], fp32, isOutput=False)
    out = nc.declare_dram_parameter("out", [TOK, D], fp32, isOutput=True)

    with tile.TileContext(nc) as tc:
        with (
            tc.tile_pool(name="const", bufs=1) as cpool,
            tc.tile_pool(name="big", bufs=1) as big,
            tc.tile_pool(name="w1p", bufs=2) as w1p,
            tc.tile_pool(name="xs", bufs=1) as xsp,
            tc.tile_pool(name="small", bufs=2) as sp,
        ):
            # ---- constants ----
            id4 = cpool.tile([4, 4], fp32)
            make_identity(nc, id4)
            id128 = cpool.tile([128, 128], fp32)
            make_identity(nc, id128)
            wg_sb = cpool.tile([128, KT, E], fp32)
            for k in range(KT):
                nc.sync.dma_start(wg_sb[:, k, :], wg[128 * k:128 * (k + 1), :])
            bg_sb = cpool.tile([E, 1], fp32)
            nc.sync.dma_start(bg_sb[:], bg[:, :])
            b2_sb = cpool.tile([E, D], bf16)
            nc.gpsimd.dma_start(b2_sb[:], b2[:, :])   # SWDGE casts f32->bf16
            b1_sb = cpool.tile([128, E, FT], fp32)
            for e in range(E):
                nc.sync.dma_start(b1_sb[:, e, :], b1t[e, :, :])

            # ---- resident activations ----
            xb = big.tile([128, KT, TOK], bf16)          # x^T in bf16, 16KB/part
            h_sb = big.tile([128, FT, TOK], bf16)        # full silu(x@W1[e]), 64KB
            y_acc = big.tile([128, TT, D], fp32)         # output accumulator, 32KB
            gate8 = big.tile([128, TT, 8], fp32)
            mask = big.tile([128, TT, E], fp32)
            maskT = big.tile([E, TOK], bf16)

            # ---- phase 0: gating (fp32) ----
            with tc.tile_pool(name="gps", bufs=1, space="PSUM") as gps, \
                 tc.tile_pool(name="trp", bufs=2, space="PSUM") as trp:
                pg = gps.tile([E, TOK], fp32)            # gate^T, 2 banks
                for k in range(KT):
                    xf = xsp.tile([128, TOK], fp32, bufs=2)
                    nc.sync.dma_start(xf[:], xTf[128 * k:128 * (k + 1), :])
                    for h in range(2):
                        nc.tensor.matmul(
                            pg[:, 512 * h:512 * (h + 1)],
                            wg_sb[:, k, :],
                            xf[:, 512 * h:512 * (h + 1)],
                            start=(k == 0), stop=(k == KT - 1),
                        )
                gT = big.tile([E, TOK], fp32, tag="gT")
                nc.scalar.activation(gT[:], pg[:], AF.Identity, bias=bg_sb[:])

                nc.vector.memset(gate8[:], NEG)
                for t in range(TT):
                    ptr = trp.tile([128, E], fp32, tag="ptr")
                    nc.tensor.transpose(ptr[:], gT[:, 128 * t:128 * (t + 1)], id4[:])
                    nc.vector.tensor_copy(gate8[:, t, 0:E], ptr[:])
                for t in range(TT):
                    m8 = sp.tile([128, 8], fp32, tag="m8")
                    nc.vector.max(m8[:], gate8[:, t, :])
                    nc.vector.memset(m8[:, 2:8], NEG)
                    rep = sp.tile([128, 8], fp32, tag="rep")
                    nc.vector.match_replace(rep[:], m8[:], gate8[:, t, :], NEG)
                    nc.vector.tensor_tensor(
                        mask[:, t, :], gate8[:, t, 0:E], rep[:, 0:E], ALU.is_gt)
                    pmt = trp.tile([E, 128], fp32, tag="pmt")
                    nc.tensor.transpose(pmt[:], mask[:, t, :], id128[:])
                    nc.vector.tensor_copy(maskT[:, 128 * t:128 * (t + 1)], pmt[:])

            # x^T bf16 load is deferred to here so the gate-phase DMAs
            # (wg, xf stream) aren't queued behind 16MB of prefetch.
            for k in range(KT):
                nc.sync.dma_start(xb[:, k, :], xTb[128 * k:128 * (k + 1), :])

            # ---- expert phase ----
            with tc.tile_pool(name="hps", bufs=2, space="PSUM") as hps, \
                 tc.tile_pool(name="yps", bufs=2, space="PSUM") as yps:
                # init y_acc with the (mask @ b2) term
                for t in range(TT):
                    pb = yps.tile([128, D], fp32, tag="y")
                    for h in range(2):
                        nc.tensor.matmul(
                            pb[:, 512 * h:512 * (h + 1)],
                            maskT[:, 128 * t:128 * (t + 1)],
                            b2_sb[:, 512 * h:512 * (h + 1)],
                            start=True, stop=True)
                    nc.scalar.activation(y_acc[:, t, :], pb[:], AF.Identity)

                for e in range(E):
                    w2t = big.tile([128, FT, D], bf16, tag="w2t")  # 64KB/part
                    for f in range(FT):
                        nc.sync.dma_start(
                            w2t[:, f, :], w2[e, 128 * f:128 * (f + 1), :])
                    # mm1 + silu: h^T[f,:] for all F
                    for fo in range(FO):
                        w1t = w1p.tile([128, KT, FCH], bf16, tag="w1t")
                        for k in range(KT):
                            nc.sync.dma_start(
                                w1t[:, k, :],
                                w1[e, 128 * k:128 * (k + 1),
                                   FCH * fo:FCH * (fo + 1)])
                        for fi in range(FCH // 128):
                            ft = fo * (FCH // 128) + fi
                            ph = hps.tile([128, TOK], fp32, tag="h")
                            for k in range(KT):
                                lw = w1t[:, k, 128 * fi:128 * (fi + 1)]
                                for h in range(2):
                                    nc.tensor.matmul(
                                        ph[:, 512 * h:512 * (h + 1)],
                                        lw,
                                        xb[:, k, 512 * h:512 * (h + 1)],
                                        start=(k == 0), stop=(k == KT - 1))
                            nc.scalar.activation(
                                h_sb[:, ft, :], ph[:], AF.Silu,
                                bias=b1_sb[:, e, ft:ft + 1])
                    # mm2 + masked accumulate
                    for t in range(TT):
                        py = yps.tile([128, D], fp32, tag="y")
                        for f in range(FT):
                            lh = h_sb[:, f, 128 * t:128 * (t + 1)]
                            for h in range(2):
                                nc.tensor.matmul(
                                    py[:, 512 * h:512 * (h + 1)],
                                    lh,
                                    w2t[:, f, 512 * h:512 * (h + 1)],
                                    start=(f == 0), stop=(f == FT - 1))
                        nc.vector.scalar_tensor_tensor(
                            y_acc[:, t, :], py[:], mask[:, t, e:e + 1],
                            y_acc[:, t, :], ALU.mult, ALU.add)

            for t in range(TT):
                nc.sync.dma_start(out[128 * t:128 * (t + 1), :], y_acc[:, t, :])

    nc.finalize()
    return nc


def _get_nc():
    if "nc" not in _cache:
        _cache["nc"] = _build()
    return _cache["nc"]


def kernel(x, Wg, bg, W1, b1, W2, b2):
    import ml_dtypes
    from concourse.bass_utils import run_bass_kernel_spmd

    nc = _get_nc()
    bf = ml_dtypes.bfloat16

    x = np.asarray(x, dtype=np.float32).reshape(B * S, D)
    Wg = np.asarray(Wg, dtype=np.float32)
    bg_c = np.ascontiguousarray(np.asarray(bg, np.float32).reshape(E, 1))
    W1b = np.ascontiguousarray(np.asarray(W1, np.float32)).astype(bf)
    W2b = np.ascontiguousarray(np.asarray(W2, np.float32)).astype(bf)
    # b1 [E,F] -> [E,128,FT] so each expert's per-partition bias column loads
    # with one contiguous DMA: b1t[e, p, j] = b1[e, j*128 + p]
    b1t = np.ascontiguousarray(
        np.asarray(b1, np.float32).reshape(E, FT, 128).transpose(0, 2, 1))
    b2_c = np.ascontiguousarray(np.asarray(b2, np.float32))

    in_maps = []
    for c in range(NCORES):
        xs = x[c * TOK:(c + 1) * TOK, :]          # [TOK, D]
        xT = np.ascontiguousarray(xs.T)           # [D, TOK] fp32
        in_maps.append({
            "xTf": xT,
            "xTb": xT.astype(bf),
            "wg": Wg,
            "bg": bg_c,
            "w1": W1b,
            "b1t": b1t,
            "w2": W2b,
            "b2": b2_c,
        })

    res = run_bass_kernel_spmd(nc, in_maps, core_ids=list(range(NCORES)),
                               **_cache.get("run_kwargs", {}))
    _cache["last_result"] = res
    out = np.concatenate([np.asarray(res.results[c]["out"])
                          for c in range(NCORES)], axis=0)
    return out.reshape(B, S, D).astype(np.float32)
